# revision 2
# baseline (speedup 1.0000x reference)
"""Trainium2 Bass kernel for nn_Loss_20933670601009 (gathered-prob NLL loss).

Strategy: the loss only touches 3 elements per (l, b) position (one gathered
prob from each of rule/token/reference tables), so instead of streaming the
full ~566MB of prob tensors through the cores, each core element-gathers the
1536 f32 values it needs straight from HBM with indirect DMA, then does a
handful of tiny vector ops + Ln + reductions.

Sharding: data-parallel over L_a (128 rows -> 16 rows x 8 cores, 512
positions per core). Per-core partial sums are combined on the host
(sum of 8 scalars), which together with the on-device -1/32 scaling
reproduces mean-over-batch of per-sequence sums.

Perf notes (from the HW trace of the 12-gather version):
  - SWDGE descriptor emission on the Q7/GpSimd engine costs ~994ns fixed +
    ~0.34ns/descriptor per indirect DMA.  Twelve [128,1]-offset gathers paid
    the ~1us fixed cost twelve times (~13us serialized).  One merged gather
    with a [128,12] offset AP emits all 1536 descriptors in one instruction
    (~1.5us): the HW reads out.size/offsets.size contiguous elements per
    offset, so offsets [P,12] -> out [P,12] is a true per-element gather.
  - Absolute flat offsets and validity masks are precomputed on the host
    (int32, exact) so no on-device offset arithmetic gates the gather; the
    gather is emitted as soon as the small meta DMA lands.
"""

import os
import sys

import numpy as np

for _p in ("/opt/trn_rl_repo", "/root/.axon_site/_ro/trn_rl_repo"):
    if os.path.isdir(_p) and _p not in sys.path:
        sys.path.insert(0, _p)

L_A, B = 128, 32
V_RULE, V_TOK, V_REF = 2048, 32000, 512
EPS = 1e-07
N_CORES = 8
L_SH = L_A // N_CORES            # 16 sequence rows per core
NPOS = L_SH * B                  # 512 positions per core
P = 128                          # SBUF partitions
J = NPOS // P                    # 4 positions per partition
SEG = (0, NPOS * V_RULE, NPOS * V_RULE + NPOS * V_TOK)
VS = (V_RULE, V_TOK, V_REF)
N_FLAT = NPOS * (V_RULE + V_TOK + V_REF)

_CACHE = {}


def _build():
    """Build + compile the per-core Bass module (same NEFF on all 8 cores)."""
    import concourse.bacc as bacc
    import concourse.bass as bass
    import concourse.mybir as mybir
    import concourse.tile as tile

    f32 = mybir.dt.float32
    i32 = mybir.dt.int32

    nc = bacc.Bacc(
        "TRN2",
        target_bir_lowering=False,
        debug=False,
        enable_asserts=False,
        num_devices=N_CORES,
    )

    # meta layout (int32 [128, 28]):
    #   cols  0:12  absolute flat gather offsets, component-major blocks of 4
    #               (rule|token|ref); host folds in segment base, per-position
    #               row base, and the max(gt,0) clamp
    #   cols 12:24  validity (gt >= 0) as f32 bit pattern
    #   cols 24:28  mask as f32 bit pattern
    meta_d = nc.dram_tensor("meta", [P, 28], i32, kind="ExternalInput").ap()
    flat_d = nc.dram_tensor("probs_flat", [N_FLAT, 1], f32, kind="ExternalInput").ap()
    out_d = nc.dram_tensor("out", [1, 1], f32, kind="ExternalOutput").ap()

    with tile.TileContext(nc) as tc:
        with (
            tc.tile_pool(name="sb", bufs=1) as pool,
            tc.tile_pool(name="ps", bufs=1, space="PSUM") as psum,
        ):
            meta = pool.tile([P, 28], i32)
            nc.sync.dma_start(out=meta[:], in_=meta_d[:])
            offs = meta[:, 0:12]
            vm = meta[:, 12:24].bitcast(f32)
            maskf = meta[:, 24:28].bitcast(f32)

            # one merged element-gather: 1536 descriptors in a single SWDGE
            # emission (offsets [P,12] -> out [P,12] reads 1 element/offset)
            gv = pool.tile([P, 12], f32)
            nc.gpsimd.indirect_dma_start(
                out=gv[:],
                out_offset=None,
                in_=flat_d[:],
                in_offset=bass.IndirectOffsetOnAxis(ap=offs, axis=0),
                element_offset=0,
            )

            # zero out gathered values for gt == -1, then sum the 3 components
            gm = pool.tile([P, 12], f32)
            nc.vector.tensor_mul(out=gm[:], in0=gv[:], in1=vm)
            s = pool.tile([P, J], f32)
            nc.vector.tensor_add(out=s[:], in0=gm[:, 0:4], in1=gm[:, 8:12])
            nc.vector.tensor_add(out=s[:], in0=s[:], in1=gm[:, 4:8])

            # prob += (prob < eps) * eps  (an add, not a clamp)
            t1 = pool.tile([P, J], f32)
            nc.vector.tensor_scalar(
                out=t1[:], in0=s[:], scalar1=EPS, scalar2=EPS,
                op0=mybir.AluOpType.is_lt, op1=mybir.AluOpType.mult,
            )
            nc.vector.tensor_add(out=s[:], in0=s[:], in1=t1[:])

            ln = pool.tile([P, J], f32)
            nc.scalar.activation(out=ln[:], in_=s[:], func=mybir.ActivationFunctionType.Ln)

            # masked row sums (tensor_tensor_reduce wedges the exec unit on
            # HW, so mul + reduce as separate ops)
            lm = pool.tile([P, J], f32)
            nc.vector.tensor_mul(out=lm[:], in0=ln[:], in1=maskf)
            rs = pool.tile([P, 1], f32)
            nc.vector.reduce_sum(out=rs[:], in_=lm[:], axis=mybir.AxisListType.X)

            # partition reduction via PE; weight -1/B folds negation + mean
            negw = pool.tile([P, 1], f32)
            nc.gpsimd.memset(negw[:], -1.0 / B)
            acc = psum.tile([1, 1], f32)
            nc.tensor.matmul(out=acc[:], lhsT=rs[:], rhs=negw[:], start=True, stop=True)
            res = pool.tile([1, 1], f32)
            nc.scalar.copy(out=res[:], in_=acc[:])
            nc.sync.dma_start(out=out_d[:], in_=res[:])

    nc.compile()
    return nc


def get_nc():
    if "nc" not in _CACHE:
        _CACHE["nc"] = _build()
    return _CACHE["nc"]


def make_in_maps(rule_probs, token_probs, reference_probs, ground_truth_actions, mask):
    """Shard the full inputs into 8 per-core input maps."""
    rule_probs = np.ascontiguousarray(np.asarray(rule_probs, dtype=np.float32))
    token_probs = np.ascontiguousarray(np.asarray(token_probs, dtype=np.float32))
    reference_probs = np.ascontiguousarray(np.asarray(reference_probs, dtype=np.float32))
    gt = np.asarray(ground_truth_actions, dtype=np.int32)
    mask = np.asarray(mask, dtype=np.int32)

    q = np.arange(NPOS, dtype=np.int64)

    in_maps = []
    for i in range(N_CORES):
        lo, hi = i * L_SH, (i + 1) * L_SH
        gt_sh = gt[lo:hi].reshape(NPOS, 3)
        meta = np.empty((P, 28), np.int32)
        for c in range(3):
            off = SEG[c] + q * VS[c] + np.maximum(gt_sh[:, c].astype(np.int64), 0)
            meta[:, c * 4:(c + 1) * 4] = off.reshape(P, J).astype(np.int32)
            meta[:, 12 + c * 4:12 + (c + 1) * 4] = (
                (gt_sh[:, c] >= 0).astype(np.float32).view(np.int32).reshape(P, J)
            )
        meta[:, 24:28] = (
            mask[lo:hi].reshape(NPOS).astype(np.float32).view(np.int32).reshape(P, J)
        )
        probs_flat = np.concatenate(
            [
                rule_probs[lo:hi].reshape(-1),
                token_probs[lo:hi].reshape(-1),
                reference_probs[lo:hi].reshape(-1),
            ]
        )
        in_maps.append({"meta": meta, "probs_flat": probs_flat.reshape(-1, 1)})
    return in_maps


def run(inputs, trace=False, trace_cores=None):
    """Run on the 8 NeuronCores; returns (scalar ndarray, BassKernelResults)."""
    from concourse.bass_utils import run_bass_kernel_spmd

    nc = get_nc()
    in_maps = make_in_maps(**inputs)
    res = run_bass_kernel_spmd(
        nc,
        in_maps,
        core_ids=list(range(N_CORES)),
        trace=trace,
        trace_cores=trace_cores,
    )
    total = np.float64(0.0)
    for r in res.results:
        total += np.float64(r["out"].reshape(())[()])
    return np.asarray(total, dtype=np.float32), res


def kernel(**inputs) -> np.ndarray:
    out, _ = run(inputs)
    return out
